# revision 14
# baseline (speedup 1.0000x reference)
"""Trainium2 Bass kernel: multi-edge-type GNN message passing.

out[t] = sum_l inv_sqrt_deg_l[t] * (sum_{e in type l, tgt_e = t} x[src_e]) @ W[l]

Strategy (8 NeuronCores, SPMD single program):
  - Host: per edge type, sort edges by target node; compute per-edge
    normalizer inv_sqrt_deg_l[tgt_e]; split target-node space into
    128-row tiles and assign a contiguous range of node tiles to each
    core (node/edge co-sharding => no collectives; outputs concatenate).
    Edges are also split by source-node half (src < 32768 vs >=) so that
    row gathers can use the int16-indexed dma_gather fast path.
  - Device, per (node_tile j, type l):
      * big multi-chunk dma_gather instructions stream edge source rows
        x[src_e] into an SBUF ring ([128, R*128] tiles, edge e of chunk c
        lands at partition e%128, free block c)
      * per 128-edge chunk: build scaled one-hot O[e,t] = (tgt_local[e]==t)*inv[e]
        with one DVE tensor_scalar, then matmul-accumulate
        S^T[d,t] += xg_chunk.T @ O into PSUM (segment-sum on the PE)
      * S^T (SBUF) @ W[l] accumulates over l into the output PSUM tile
  - Degree counts, rsqrt, sorting are index-side host preprocessing;
    all embedding data movement and FLOPs run on device.
"""

import numpy as np
from contextlib import ExitStack

import concourse.bass as bass
import concourse.tile as tile
from concourse import bacc, mybir
from concourse.bass_utils import run_bass_kernel_spmd

P = 128
D = 128
N_CORES = 8
SPLIT = 32768          # int16 table split
RCH = 16               # max chunks per dma_gather instruction

F32 = mybir.dt.float32
I16 = mybir.dt.int16

# test.py pokes this to get at profiling results of the last run
last_run_results = None


class Layout:
    """Uniform (core-independent) chunk/gather layout."""

    def __init__(self, n_nodes, L, J):
        self.n_nodes = n_nodes
        self.L = L
        self.J = J
        # per (l, half, j): padded chunk count
        self.C = np.zeros((L, 2, J), dtype=np.int64)
        # filled by finalize():
        self.tile_chunks = None   # [l][j] -> list of global chunk ids
        self.chunk_meta = None    # gid -> (gather id, offset in gather)
        self.gathers = None       # list of (l, h, idxcol0, nch)
        self.NCH = 0
        self.NI = 0

    def finalize(self):
        L, J = self.L, self.J
        self.tile_chunks = [[[] for _ in range(J)] for _ in range(L)]
        self.chunk_meta = {}
        self.gathers = []
        self.seg_colbase = {}
        gid = 0
        icol = 0
        for l in range(L):
            for h in range(2):
                # contiguous chunk stream for this (l, h); break into gathers
                seg_gids = []
                for j in range(J):
                    for _ in range(int(self.C[l, h, j])):
                        self.tile_chunks[l][j].append(gid)
                        seg_gids.append(gid)
                        gid += 1
                self.seg_colbase[(l, h)] = icol
                for c0 in range(0, len(seg_gids), RCH):
                    nch = min(RCH, len(seg_gids) - c0)
                    gi = len(self.gathers)
                    self.gathers.append((l, h, icol + c0 * 8, nch))
                    for k in range(nch):
                        self.chunk_meta[seg_gids[c0 + k]] = (gi, k)
                icol += len(seg_gids) * 8
        self.NCH = gid
        self.NI = icol


def _preprocess(adjacency, n_nodes, n_cores):
    """Sort/shard edges. Returns (layout, per_core (idx16, meta-parts))."""
    L = adjacency.shape[0]
    tiles_total = -(-n_nodes // P)
    J = -(-tiles_total // n_cores)

    per_type = []
    for l in range(L):
        src = np.asarray(adjacency[l, :, 0], dtype=np.int64)
        tgt = np.asarray(adjacency[l, :, 1], dtype=np.int64)
        deg = np.bincount(tgt, minlength=n_nodes)
        inv = (1.0 / np.sqrt(np.maximum(deg, 1.0))).astype(np.float32)
        order = np.argsort(tgt, kind="stable")
        srcs = src[order]
        tgts = tgt[order]
        inve = inv[tgts]
        bounds = np.searchsorted(tgts, np.arange(tiles_total + 1) * P)
        per_type.append((srcs, tgts, inve, bounds))

    # per (core, l, j): edge lists split by src half, sorted by src
    edges = {}
    lay = Layout(n_nodes, L, J)
    for l in range(L):
        srcs, tgts, inve, bounds = per_type[l]
        for c in range(n_cores):
            for j in range(J):
                t = c * J + j
                if t >= tiles_total:
                    continue
                lo, hi = int(bounds[t]), int(bounds[t + 1])
                s = srcs[lo:hi]
                tl = (tgts[lo:hi] - t * P).astype(np.float32)
                iv = inve[lo:hi]
                o = np.argsort(s, kind="stable")
                s, tl, iv = s[o], tl[o], iv[o]
                cut = int(np.searchsorted(s, SPLIT))
                edges[(c, l, 0, j)] = (s[:cut], tl[:cut], iv[:cut])
                edges[(c, l, 1, j)] = (s[cut:] - SPLIT, tl[cut:], iv[cut:])
                lay.C[l, 0, j] = max(lay.C[l, 0, j], -(-cut // P))
                lay.C[l, 1, j] = max(lay.C[l, 1, j], -(-(len(s) - cut) // P))
        # every (l, j) must have >= 1 chunk so the PSUM group is non-empty
        for j in range(J):
            if lay.C[l, 0, j] + lay.C[l, 1, j] == 0:
                lay.C[l, 0, j] = 1
    lay.finalize()

    NCH, NI = lay.NCH, lay.NI
    per_core = []
    for c in range(n_cores):
        idx16 = np.zeros((128, NI), np.int16)
        srcidx = np.zeros((P, NCH), np.int32)
        tgtl = np.full((P, NCH), -1.0, np.float32)
        invv = np.zeros((P, NCH), np.float32)
        for l in range(L):
            for h in range(2):
                colbase = lay.seg_colbase[(l, h)]
                # rebuild this (l,h) segment's padded edge stream
                seg_idx = []
                for j in range(J):
                    nch = int(lay.C[l, h, j])
                    if nch == 0:
                        continue
                    key = (c, l, h, j)
                    s, tl, iv = edges.get(key, (np.zeros(0, np.int64),
                                                np.zeros(0, np.float32),
                                                np.zeros(0, np.float32)))
                    n = len(s)
                    npad = nch * P
                    sblk = np.zeros(npad, np.int64)
                    sblk[:n] = s
                    tblk = np.full(npad, -1.0, np.float32)
                    tblk[:n] = tl
                    iblk = np.zeros(npad, np.float32)
                    iblk[:n] = iv
                    # meta columns for these chunks
                    g0 = lay.tile_chunks[l][j][0]
                    # find position of this (l,h,j)'s chunks within tile list
                    # simpler: chunk ids are consecutive within (l,h) stream:
                    seg_idx.append(sblk)
                    # meta: chunk ids for (l,h,j) are the stream ids; compute:
                    # they were appended in order, so reconstruct below.
                for j2 in range(J):
                    pass
                if seg_idx:
                    allidx = np.concatenate(seg_idx)
                else:
                    allidx = np.zeros(0, np.int64)
                n16 = len(allidx) // 16
                if n16:
                    w = allidx.astype(np.int16).reshape(n16, 16).T  # [16, n16]
                    idx16[:, colbase:colbase + n16] = np.tile(w, (8, 1))
        # meta (tgtl / inv) by global chunk id
        for l in range(L):
            for j in range(J):
                ids = lay.tile_chunks[l][j]
                pos = 0
                for h in range(2):
                    nch = int(lay.C[l, h, j])
                    if nch == 0:
                        continue
                    key = (c, l, h, j)
                    s, tl, iv = edges.get(key, (np.zeros(0, np.int64),
                                                np.zeros(0, np.float32),
                                                np.zeros(0, np.float32)))
                    n = len(s)
                    npad = nch * P
                    tblk = np.full(npad, -1.0, np.float32)
                    tblk[:n] = tl
                    iblk = np.zeros(npad, np.float32)
                    iblk[:n] = iv
                    sblk = np.zeros(npad, np.int64)
                    sblk[:n] = s + (SPLIT if h == 1 else 0)
                    for k in range(nch):
                        g = ids[pos + k]
                        tgtl[:, g] = tblk[k * P:(k + 1) * P]
                        invv[:, g] = iblk[k * P:(k + 1) * P]
                        srcidx[:, g] = sblk[k * P:(k + 1) * P].astype(np.int32)
                    pos += nch
        per_core.append((srcidx, tgtl, invv))
    return lay, per_core


def _build_program(lay):
    # meta layout (f32): [0,NCH) tgt_local | [NCH,2NCH) inv | iota P | W L*D
    L, J, NCH, NI = lay.L, lay.J, lay.NCH, lay.NI
    n_nodes = lay.n_nodes
    M = 2 * NCH + P + L * D
    nc = bacc.Bacc("TRN2")
    emb = nc.declare_dram_parameter("emb", [n_nodes, D], F32, isOutput=False)
    idx_d = nc.declare_dram_parameter("srcidx", [P, NCH], mybir.dt.int32, isOutput=False)
    meta_d = nc.declare_dram_parameter("meta", [P, M], F32, isOutput=False)
    out_d = nc.declare_dram_parameter("out", [J * P, D], F32, isOutput=True)

    with tile.TileContext(nc) as tc, ExitStack() as ctx:
        const = ctx.enter_context(tc.tile_pool(name="const", bufs=1))
        xgp = ctx.enter_context(tc.tile_pool(name="xg", bufs=8))
        ohp = ctx.enter_context(tc.tile_pool(name="oh", bufs=6))
        stp = ctx.enter_context(tc.tile_pool(name="stsb", bufs=3))
        outp = ctx.enter_context(tc.tile_pool(name="osb", bufs=3))
        psum1 = ctx.enter_context(tc.tile_pool(name="ps1", bufs=2, space="PSUM"))
        psum2 = ctx.enter_context(tc.tile_pool(name="ps2", bufs=2, space="PSUM"))

        idx_sb = const.tile([P, NCH], mybir.dt.int32)
        nc.sync.dma_start(idx_sb[:], idx_d[:])
        meta_sb = const.tile([P, M], F32)
        nc.sync.dma_start(meta_sb[:], meta_d[:])


        for j in range(J):
            opsum = psum2.tile([P, D], F32, tag="opsum")
            for l in range(L):
                ids = lay.tile_chunks[l][j]
                st_ps = psum1.tile([P, P], F32, tag="st")
                for k, g in enumerate(ids):
                    gt = xgp.tile([P, D], F32, tag="xg")
                    nc.gpsimd.indirect_dma_start(
                        out=gt[:],
                        out_offset=None,
                        in_=emb[:],
                        in_offset=bass.IndirectOffsetOnAxis(
                            ap=idx_sb[:, g:g + 1], axis=0
                        ),
                    )
                    oh = ohp.tile([P, P], F32, tag="oh")
                    nc.vector.tensor_scalar(
                        out=oh[:],
                        in0=meta_sb[:, 2 * NCH:2 * NCH + P],
                        scalar1=meta_sb[:, g:g + 1],
                        scalar2=meta_sb[:, NCH + g:NCH + g + 1],
                        op0=mybir.AluOpType.is_equal,
                        op1=mybir.AluOpType.mult,
                    )
                    nc.tensor.matmul(
                        out=st_ps[:],
                        lhsT=gt[:],
                        rhs=oh[:],
                        start=(k == 0),
                        stop=(k == len(ids) - 1),
                    )
                st_sb = stp.tile([P, P], F32, tag="stsb")
                nc.scalar.copy(st_sb[:], st_ps[:])
                nc.tensor.matmul(
                    out=opsum[:],
                    lhsT=st_sb[:],
                    rhs=meta_sb[:, 2 * NCH + P + l * D:2 * NCH + P + (l + 1) * D],
                    start=(l == 0),
                    stop=(l == L - 1),
                )
            osb = outp.tile([P, D], F32, tag="osb")
            nc.scalar.copy(osb[:], opsum[:])
            nc.sync.dma_start(out_d[j * P:(j + 1) * P, :], osb[:])
    nc.compile()
    return nc


def _run(node_embeddings, adjacency, W, n_cores=N_CORES, **run_kwargs):
    global last_run_results
    node_embeddings = np.ascontiguousarray(np.asarray(node_embeddings, dtype=np.float32))
    adjacency = np.asarray(adjacency, dtype=np.int32)
    W = np.asarray(W, dtype=np.float32)
    n_nodes = node_embeddings.shape[0]
    L = adjacency.shape[0]

    lay, per_core = _preprocess(adjacency, n_nodes, n_cores)
    nc = _build_program(lay)

    w_cat = np.concatenate([W[l] for l in range(L)], axis=1).astype(np.float32)
    iotaf = np.tile(np.arange(P, dtype=np.float32), (P, 1))
    in_maps = [
        dict(
            emb=node_embeddings,
            srcidx=srcidx,
            meta=np.ascontiguousarray(
                np.concatenate([tg, iv, iotaf, w_cat], axis=1), dtype=np.float32
            ),
        )
        for (srcidx, tg, iv) in per_core
    ]
    res = run_bass_kernel_spmd(nc, in_maps, core_ids=list(range(n_cores)), **run_kwargs)
    last_run_results = res
    outs = [res.results[c]["out"] for c in range(n_cores)]
    full = np.concatenate(outs, axis=0)[:n_nodes]
    return np.ascontiguousarray(full, dtype=np.float32)


def kernel(node_embeddings, adjacency, W):
    return _run(node_embeddings, adjacency, W)
